# revision 1
# baseline (speedup 1.0000x reference)
"""NodeAttention (GNN scatter-softmax attention) on 8 Trainium2 NeuronCores.

Strategy:
- Host deals nodes to 8 cores round-robin by degree rank, so every core sees a
  near-identical degree profile; one static NEFF serves all cores (SPMD).
- Per core: 49 node-tiles x 128 nodes; node-tile t gets a dense padded slot
  grid [128, D_t] (D_t = max degree in tile across cores; ~3% padding).
- No gather at all: the host replicates x per SLOT (xTc column per edge slot,
  inverse-permuted so the device's KV build lands row r at slot order
  r = tile_base + p*D + k). The device builds the slot-ordered bf16 KV=(K|V)
  table in DRAM via matmuls, and each node-tile reads its KV rows back with a
  plain full-rate DMA. Build and edge phases are interleaved per 4096-row
  batch so compute starts ~immediately.
- Per-edge scores: bf16 QK muls on DVE, per-edge bias via block-diagonal
  matmuls (3 slots x 34 ef-features stacked on 102 partitions), exp on ACT,
  softmax normalization AFTER aggregation (denominator constant within a
  node's slots), projection + residual on PE/DVE, layernorm batched at the
  end (single Sqrt table load).
- No max-subtraction in softmax (scores are O(10); identical result).
  Padding slots masked via an extra edge-feature column (weight 1, value -75).
- temp/sqrt(d) folded into Wq; temp folded into We; be via a ones column.
"""

import os
import numpy as np
import ml_dtypes

import concourse.bass as bass
import concourse.bacc as bacc
import concourse.tile as tile
from concourse import mybir
from concourse.bass_utils import run_bass_kernel_spmd
from concourse.masks import make_identity

N, E = 50000, 800000
D_NODE, D_EDGE, H = 64, 32, 4
D_H = D_NODE // H
LN_EPS = 1e-5
NCORES = 8
P = 128
NT = 49                # node tiles per core
NPC = NT * P           # padded nodes per core = 6272
KB = 4096              # KV-build rows per DMA batch
KJ = KB // P           # rows per partition per batch
EF_R = 34              # 32 ef features + mask col + ones col (carries be)
EF3 = 3 * EF_R         # 102: three slots stacked on partitions
MASK_VAL = -75.0
F32 = mybir.dt.float32
BF16 = mybir.dt.bfloat16
BF_NP = ml_dtypes.bfloat16


def _col_of_row(r):
    """Inverse of the KV build's column->row permutation.  Build position
    (b, j, p) takes xTc column b*KB + j*P + p to table row b*KB + p*KJ + j
    (per-partition-contiguous DMA writes).  Given the desired row, return
    the column to place the source vector at."""
    b = r // KB
    w = r % KB
    return b * KB + (w % KJ) * P + (w // KJ)


# ---------------------------------------------------------------- host prep --
def _host_prep(node_features, edge_features, edge_index, Wq, bq, Wk, bk, Wv, bv,
               We, be, Wo, bo, ln_gamma, ln_beta, log_temp):
    x = np.ascontiguousarray(np.asarray(node_features, dtype=np.float32))
    ef = np.ascontiguousarray(np.asarray(edge_features, dtype=np.float32))
    src = np.asarray(edge_index[0], dtype=np.int64)
    tgt = np.asarray(edge_index[1], dtype=np.int64)
    temp = np.exp(np.asarray(log_temp, dtype=np.float32))

    deg = np.bincount(tgt, minlength=N)
    order = np.argsort(-deg, kind="stable")
    node_lists = []
    for c in range(NCORES):
        nl = order[c::NCORES]
        nl = np.concatenate([nl, np.full(NPC - len(nl), -1, dtype=np.int64)])
        node_lists.append(nl)

    D_t = np.zeros(NT, dtype=np.int64)
    for c in range(NCORES):
        d = np.where(node_lists[c] >= 0, deg[np.maximum(node_lists[c], 0)], 0)
        D_t = np.maximum(D_t, d.reshape(NT, P).max(axis=1))
    D_t = np.maximum(D_t, 1)
    assert D_t.max() <= 128, f"degree {D_t.max()} exceeds single-bank design"
    SD = int(D_t.sum())
    KC_t = [-(-int(d) // 3) for d in D_t]
    TOT = -(-SD * P // KB) * KB          # padded table rows

    eorder = np.argsort(tgt, kind="stable")
    estart = np.zeros(N + 1, dtype=np.int64)
    np.cumsum(deg, out=estart[1:])

    qscale = (np.repeat(temp, D_H) / np.sqrt(D_H)).astype(np.float32)
    Wq_aug = (np.concatenate([np.asarray(Wq).T, np.asarray(bq)[None, :]], 0)
              * qscale[None, :]).astype(BF_NP)                           # [65,64]
    Wkv_aug = np.concatenate(
        [np.concatenate([np.asarray(Wk).T, np.asarray(Wv).T], 1),
         np.concatenate([np.asarray(bk), np.asarray(bv)])[None, :]], 0
    ).astype(BF_NP)                                                      # [65,128]
    We_augT = np.concatenate(
        [np.asarray(We).T * temp[None, :],
         np.ones((1, H), np.float32),
         (np.asarray(be) * temp)[None, :]], 0
    ).astype(np.float32)                                                 # [34,4]
    We_blk = np.zeros((EF3, 3 * H), dtype=np.float32)
    for j3 in range(3):
        We_blk[j3 * EF_R:(j3 + 1) * EF_R, j3 * H:(j3 + 1) * H] = We_augT
    We_blk = We_blk.astype(BF_NP)
    Wo_aug = np.concatenate(
        [np.asarray(Wo).T, np.asarray(bo)[None, :]], 0).astype(np.float32)  # [65,64]
    gb = np.stack([np.asarray(ln_gamma), np.asarray(ln_beta)]).astype(np.float32)

    x_aug = np.concatenate(
        [x, np.ones((N, 1), np.float32)], 1).astype(BF_NP)               # [N,65]

    SKC = sum(KC_t)
    per_core = []
    for c in range(NCORES):
        nl = node_lists[c]
        efT = np.zeros((EF3, SKC * P), dtype=BF_NP)
        xTc = np.zeros((65, TOT), dtype=BF_NP)
        doff = 0
        koff = 0
        for t in range(NT):
            D = int(D_t[t])
            KC = KC_t[t]
            nlt = nl[t * P:(t + 1) * P]
            degt = np.where(nlt >= 0, deg[np.maximum(nlt, 0)], 0)
            k = np.arange(D)
            valid = k[None, :] < degt[:, None]                    # [P,D]
            pos = estart[np.maximum(nlt, 0)][:, None] + k[None, :]
            eids = eorder[np.minimum(pos, E - 1)]
            eids = np.where(valid, eids, 0)
            gsrc = np.where(valid, src[eids], 0)                  # [P,D]
            # slot (p, k) lives at table row doff*P + p*D + k
            rows = doff * P + (np.arange(P)[:, None] * D + k[None, :])
            cols = _col_of_row(rows)
            xTc[:, cols.ravel()] = x_aug[gsrc.ravel()].T
            blk = np.zeros((P, KC * 3, EF_R), dtype=np.float32)
            blk[:, :, D_EDGE] = MASK_VAL
            blk[:, :D, :D_EDGE] = np.where(valid[:, :, None], ef[eids], 0.0)
            blk[:, :D, D_EDGE] = np.where(valid, 0.0, MASK_VAL)
            blk[:, :, D_EDGE + 1] = 1.0
            # [P, KC, 3, EF_R] -> [3, EF_R, KC, P] -> [102, KC*128]
            efT[:, koff * P:(koff + KC) * P] = (
                blk.reshape(P, KC, 3, EF_R).transpose(2, 3, 1, 0)
                .reshape(EF3, KC * P).astype(BF_NP))
            doff += D
            koff += KC
        xq = np.where(nl[:, None] >= 0, x[np.maximum(nl, 0)], 0.0).astype(np.float32)
        xqT_aug = np.concatenate([xq.T, np.ones((1, NPC), np.float32)],
                                 0).astype(BF_NP)
        xq_g = np.ascontiguousarray(
            xq.reshape(NT, P, D_NODE).transpose(1, 0, 2).reshape(P, NT * D_NODE))
        per_core.append({
            "efT": efT,
            "xTc": xTc,
            "xqT": np.ascontiguousarray(xqT_aug),
            "xq": xq_g,
            "wq": Wq_aug,
            "wkv": np.ascontiguousarray(Wkv_aug),
            "we": np.ascontiguousarray(We_blk),
            "wo": Wo_aug,
            "gb": gb,
        })
    meta = dict(D_seq=[int(d) for d in D_t], TOT=TOT)
    return per_core, node_lists, meta


# ------------------------------------------------------------- bass kernel --
def _build_kernel(meta, debug_mode=None):
    if debug_mode is None:
        debug_mode = os.environ.get("KERNEL_DEBUG_MODE", "")
    D_seq = meta["D_seq"]
    TOT = meta["TOT"]
    SD = sum(D_seq)
    KC_seq = [-(-d // 3) for d in D_seq]
    SKC = sum(KC_seq)
    # eft groups: ~5 DMAs over the run, aligned to tile KC blocks
    NG = 5
    tgt_sz = -(-SKC // NG)
    gsz = []
    acc = 0
    for kc in KC_seq:
        if acc + kc > tgt_sz and acc > 0:
            gsz.append(acc)
            acc = 0
        acc += kc
    gsz.append(acc)
    nc = bacc.Bacc(None, target_bir_lowering=False)

    efT = nc.dram_tensor("efT", [EF3, SKC * P], BF16, kind="ExternalInput")
    xTc = nc.dram_tensor("xTc", [65, TOT], BF16, kind="ExternalInput")
    xqT = nc.dram_tensor("xqT", [65, NPC], BF16, kind="ExternalInput")
    xq = nc.dram_tensor("xq", [P, NT * D_NODE], F32, kind="ExternalInput")
    wq = nc.dram_tensor("wq", [65, D_NODE], BF16, kind="ExternalInput")
    wkv = nc.dram_tensor("wkv", [65, 2 * D_NODE], BF16, kind="ExternalInput")
    we = nc.dram_tensor("we", [EF3, 3 * H], BF16, kind="ExternalInput")
    wo = nc.dram_tensor("wo", [65, D_NODE], F32, kind="ExternalInput")
    gb = nc.dram_tensor("gb", [2, D_NODE], F32, kind="ExternalInput")
    y = nc.dram_tensor("y", [P, NT * D_NODE], F32, kind="ExternalOutput")

    with tile.TileContext(nc) as tc:
        with (
            tc.tile_pool(name="dram", bufs=1, space="DRAM") as dpool,
            tc.tile_pool(name="singles", bufs=1) as singles,
        ):
            kv = dpool.tile([TOT, 2 * D_NODE], BF16)

            wq_sb = singles.tile([65, D_NODE], BF16)
            nc.sync.dma_start(out=wq_sb[:], in_=wq[:])
            wkv_sb = singles.tile([65, 2 * D_NODE], BF16)
            nc.sync.dma_start(out=wkv_sb[:], in_=wkv[:])
            we_sb = singles.tile([EF3, 3 * H], BF16)
            nc.sync.dma_start(out=we_sb[:], in_=we[:])
            wo_sb = singles.tile([65, D_NODE], F32)
            nc.sync.dma_start(out=wo_sb[:], in_=wo[:])
            gamma_sb = singles.tile([P, D_NODE], F32)
            nc.sync.dma_start(
                out=gamma_sb[:],
                in_=bass.AP(tensor=gb[:].tensor, offset=0,
                            ap=[[0, P], [1, D_NODE]]))
            beta_sb = singles.tile([P, D_NODE], F32)
            nc.sync.dma_start(
                out=beta_sb[:],
                in_=bass.AP(tensor=gb[:].tensor, offset=D_NODE,
                            ap=[[0, P], [1, D_NODE]]))
            xqT_sb = singles.tile([65, NPC], BF16)
            nc.sync.dma_start(out=xqT_sb[:], in_=xqT[:])
            xq_sb = singles.tile([P, NT, D_NODE], F32)
            nc.sync.dma_start(out=xq_sb[:], in_=xq[:])
            ident = singles.tile([P, P], F32)
            make_identity(nc, ident[:])
            ones_sb = singles.tile([1, P], F32)
            nc.vector.memset(ones_sb[:], 1.0)
            wob_sb = singles.tile([1, D_NODE], F32)
            nc.sync.dma_start(out=wob_sb[:], in_=wo[64:65, :])
            eps_sb = singles.tile([P, 1], F32)
            nc.vector.memset(eps_sb[:], LN_EPS)
            yout_sb = singles.tile([P, NT, D_NODE], F32)
            mv_sb = singles.tile([P, NT, 2], F32)

            with (
                tc.tile_pool(name="kvb", bufs=3) as kvb,
                tc.tile_pool(name="kvp", bufs=2, space="PSUM") as kvp,
                tc.tile_pool(name="kvg", bufs=2) as kvgp,
                tc.tile_pool(name="eft", bufs=2) as eftp,
                tc.tile_pool(name="mid", bufs=2) as midp,
                tc.tile_pool(name="sml", bufs=3) as smlp,
                tc.tile_pool(name="pq", bufs=1, space="PSUM") as pq,
                tc.tile_pool(name="pb", bufs=2, space="PSUM") as pb,
                tc.tile_pool(name="pt", bufs=1, space="PSUM") as ptp,
                tc.tile_pool(name="py", bufs=2, space="PSUM") as pyp,
            ):
                def build_batch(b):
                    xt_sb = kvb.tile([65, KB], BF16, name="xt_sb")
                    nc.sync.dma_start(
                        out=xt_sb[:], in_=xTc[:, b * KB:(b + 1) * KB])
                    kv_sb = kvb.tile([P, KJ, 2 * D_NODE], BF16, name="kv_sb")
                    for jj in range(KB // 512):
                        pt = kvp.tile([P, 4, 2 * D_NODE], F32, name="pt")
                        for j4 in range(4):
                            j = jj * 4 + j4
                            nc.tensor.matmul(
                                out=pt[:, j4, :],
                                lhsT=xt_sb[:, j * P:(j + 1) * P],
                                rhs=wkv_sb[:], start=True, stop=True)
                        nc.scalar.copy(
                            out=kv_sb[:, jj * 4:(jj + 1) * 4, :], in_=pt[:])
                    # build pos (b, j, p) -> row b*KB + p*KJ + j: per
                    # partition KJ consecutive rows -> contiguous 2KB
                    nc.sync.dma_start(
                        out=bass.AP(
                            tensor=kv[:].tensor,
                            offset=kv[:].offset + b * KB * 2 * D_NODE,
                            ap=[[KJ * 2 * D_NODE, P],
                                [2 * D_NODE, KJ],
                                [1, 2 * D_NODE]]),
                        in_=kv_sb[:])

                built = 0            # batches emitted
                NB = TOT // KB
                doff = 0
                koff = 0
                goff = 0
                gi = 0
                gleft = 0
                eft_sb = None
                for t in range(NT):
                    D = D_seq[t]
                    KC = KC_seq[t]
                    # ensure this tile's kv rows are built
                    need = -(-((doff + D) * P) // KB)
                    while built < min(need, NB):
                        build_batch(built)
                        built += 1
                    if debug_mode == "kv":
                        z = smlp.tile([P, D_NODE], F32, tag="y3", name="z")
                        nc.vector.memset(z[:], 0.0)
                        nc.sync.dma_start(
                            out=y[:, t * D_NODE:(t + 1) * D_NODE], in_=z[:])
                        doff += D
                        koff += KC
                        continue
                    # slot (p, k) at row doff*P + p*D + k: per partition D
                    # consecutive 256B rows -> contiguous
                    kvg = kvgp.tile([P, D, 2 * D_NODE], BF16, tag="kvg",
                                    name="kvg")
                    # issue table reads from the ACT HWDGE queue: keeps the
                    # SP sequencer (build writes + xt loads) off the critical
                    # path of the edge phase
                    nc.scalar.dma_start(
                        out=kvg[:],
                        in_=bass.AP(
                            tensor=kv[:].tensor,
                            offset=kv[:].offset + doff * P * 2 * D_NODE,
                            ap=[[D * 2 * D_NODE, P],
                                [2 * D_NODE, D],
                                [1, 2 * D_NODE]]))
                    if gleft == 0:
                        skc = gsz[gi]
                        eft_sb = eftp.tile([EF3, skc, P], BF16, tag="eft",
                                           name="eft_sb")
                        nc.sync.dma_start(
                            out=eft_sb[:], in_=efT[:, goff * P:(goff + skc) * P])
                        gbase = goff
                        goff += skc
                        gi += 1
                        gleft = skc
                    kbase = koff - gbase

                    # Q' (temp/sqrt(dh) folded) for this tile's 128 nodes
                    qp = pq.tile([P, D_NODE], F32, tag="qp", name="qp")
                    nc.tensor.matmul(out=qp[:], lhsT=xqT_sb[:, t * P:(t + 1) * P],
                                     rhs=wq_sb[:], start=True, stop=True)
                    q_sb = smlp.tile([P, D_NODE], BF16, tag="q", name="q_sb")
                    nc.scalar.copy(out=q_sb[:], in_=qp[:])

                    # per-edge bias: 3 slots per matmul via block-diagonal We
                    biasp = pb.tile([P, 3 * KC, H], F32, tag="biasp", name="biasp")
                    for k in range(KC):
                        nc.tensor.matmul(out=biasp[:, 3 * k:3 * (k + 1), :],
                                         lhsT=eft_sb[:, kbase + k, :],
                                         rhs=we_sb[:], start=True, stop=True)

                    # scores
                    qkp = midp.tile([P, D, H, D_H], BF16, tag="qkp", name="qkp")
                    q_b = bass.AP(tensor=q_sb[:].tensor, offset=q_sb[:].offset,
                                  ap=[q_sb[:].ap[0], [0, D], [1, D_NODE]])
                    nc.vector.tensor_mul(
                        out=qkp[:].rearrange("p d h w -> p d (h w)"),
                        in0=kvg[:, :, 0:D_NODE], in1=q_b)
                    sc = smlp.tile([P, D, H], F32, tag="sc", name="sc")
                    nc.vector.tensor_reduce(
                        out=sc[:], in_=qkp[:], axis=mybir.AxisListType.X,
                        op=mybir.AluOpType.add)
                    sc2 = smlp.tile([P, D, H], F32, tag="sc2", name="sc2")
                    nc.vector.tensor_add(out=sc2[:], in0=sc[:],
                                         in1=biasp[:, 0:D, :])
                    ex = smlp.tile([P, D, H], BF16, tag="ex", name="ex")
                    nc.scalar.activation(out=ex[:], in_=sc2[:],
                                         func=mybir.ActivationFunctionType.Exp)

                    den = smlp.tile([P, H], F32, tag="den", name="den")
                    nc.vector.tensor_reduce(
                        out=den[:], in_=ex[:].rearrange("p d h -> p h d"),
                        axis=mybir.AxisListType.X, op=mybir.AluOpType.add)
                    rden = smlp.tile([P, H], F32, tag="rden", name="rden")
                    nc.vector.tensor_scalar_add(den[:], den[:], 1e-10)
                    nc.vector.reciprocal(out=rden[:], in_=den[:])

                    exv = midp.tile([P, D, H, D_H], BF16, tag="exv", name="exv")
                    nc.vector.tensor_mul(
                        out=exv[:],
                        in0=kvg[:, :, D_NODE:2 * D_NODE].rearrange(
                            "p d (h w) -> p d h w", h=H),
                        in1=ex[:].to_broadcast([P, D, H, D_H]))
                    unn = smlp.tile([P, H, D_H], F32, tag="unn", name="unn")
                    nc.vector.tensor_reduce(
                        out=unn[:], in_=exv[:].rearrange("p d h w -> p h w d"),
                        axis=mybir.AxisListType.X, op=mybir.AluOpType.add)
                    outn = smlp.tile([P, H, D_H], F32, tag="outn", name="outn")
                    nc.vector.tensor_mul(
                        out=outn[:], in0=unn[:],
                        in1=rden[:].to_broadcast([P, H, D_H]))

                    # projection: y1 = outn @ Wo.T + bo + xq
                    tp = ptp.tile([D_NODE, P], F32, tag="tp", name="tp")
                    nc.tensor.transpose(
                        out=tp[:], in_=outn[:].rearrange("p h w -> p (h w)"),
                        identity=ident[:])
                    tps = smlp.tile([D_NODE, P], F32, tag="tps", name="tps")
                    nc.scalar.copy(out=tps[:], in_=tp[:])
                    yp = pyp.tile([P, D_NODE], F32, tag="yp", name="yp")
                    nc.tensor.matmul(out=yp[:], lhsT=tps[:], rhs=wo_sb[0:64, :],
                                     start=True, stop=False)
                    nc.tensor.matmul(out=yp[:], lhsT=ones_sb[:], rhs=wob_sb[:],
                                     start=False, stop=True)
                    nc.vector.tensor_add(out=yout_sb[:, t, :], in0=yp[:],
                                         in1=xq_sb[:, t, :])
                    stats = smlp.tile([P, 6], F32, tag="stats", name="stats")
                    nc.vector.bn_stats(out=stats[:], in_=yout_sb[:, t, :])
                    nc.vector.bn_aggr(out=mv_sb[:, t, :], in_=stats[:])
                    doff += D
                    koff += KC
                    gleft -= KC

                if not debug_mode:
                    # ---- batched layernorm epilogue ----
                    mu = bass.AP(tensor=mv_sb[:].tensor, offset=mv_sb[:].offset,
                                 ap=[mv_sb[:].ap[0], [2, NT]])
                    var = bass.AP(tensor=mv_sb[:].tensor,
                                  offset=mv_sb[:].offset + 1,
                                  ap=[mv_sb[:].ap[0], [2, NT]])
                    sd_sb = singles.tile([P, NT], F32)
                    nc.scalar.activation(out=sd_sb[:], in_=var,
                                         func=mybir.ActivationFunctionType.Sqrt,
                                         bias=eps_sb[:])
                    rsd_sb = singles.tile([P, NT], F32)
                    nc.vector.reciprocal(out=rsd_sb[:], in_=sd_sb[:])
                    mursd_sb = singles.tile([P, NT], F32)
                    nc.vector.tensor_mul(out=mursd_sb[:], in0=mu, in1=rsd_sb[:])

                    def bc_t(a):   # [P, NT] -> [P, NT, 64] (bcast feature)
                        return bass.AP(tensor=a.tensor, offset=a.offset,
                                       ap=list(a.ap) + [[0, D_NODE]])

                    def bc_f(a):   # [P, 64] -> [P, NT, 64] (bcast tile)
                        return bass.AP(tensor=a.tensor, offset=a.offset,
                                       ap=[a.ap[0], [0, NT], a.ap[1]])

                    nc.vector.tensor_mul(out=yout_sb[:], in0=yout_sb[:],
                                         in1=bc_t(rsd_sb[:]))
                    nc.vector.tensor_sub(out=yout_sb[:], in0=yout_sb[:],
                                         in1=bc_t(mursd_sb[:]))
                    nc.vector.tensor_mul(out=yout_sb[:], in0=yout_sb[:],
                                         in1=bc_f(gamma_sb[:]))
                    nc.vector.tensor_add(out=yout_sb[:], in0=yout_sb[:],
                                         in1=bc_f(beta_sb[:]))
                    nc.sync.dma_start(out=y[:], in_=yout_sb[:])

    nc.compile()
    return nc


# ------------------------------------------------------------------ driver --
def kernel(**inputs) -> np.ndarray:
    per_core, node_lists, meta = _host_prep(**inputs)
    nc = _build_kernel(meta)
    res = run_bass_kernel_spmd(nc, per_core, core_ids=list(range(NCORES)))
    y_full = np.zeros((N, D_NODE), dtype=np.float32)
    for c in range(NCORES):
        yc = res.results[c]["y"].reshape(P, NT, D_NODE).transpose(1, 0, 2)
        yc = yc.reshape(NPC, D_NODE)
        nl = node_lists[c]
        real = nl >= 0
        y_full[nl[real]] = yc[real]
    return y_full



# revision 9
# speedup vs baseline: 1.8673x; 1.8673x over previous
"""NodeAttention (GNN scatter-softmax attention) on 8 Trainium2 NeuronCores.

Strategy (v2 — no DRAM KV roundtrip):
- Host deals nodes to 8 cores round-robin by degree rank; one static NEFF
  serves all cores (SPMD).  Per core: 49 node-tiles x 128 nodes; tile t gets a
  dense slot grid [128, D_t] (D_t = max degree in tile across cores, padded to
  even).
- The host replicates x per SLOT in contraction-major order (xt stream); the
  device builds K|V for 16 slot-columns at a time straight into PSUM via
  matmuls.  Pool multiplies Q against the K half directly from PSUM (no K
  copy); ACT copies the V half to SBUF feature-major.
- Score d-reduction and the V k-aggregation use contiguous halving-add trees
  on DVE (2x bf16 mode) instead of 1x tensor_reduce.
- Per-edge bias (ef @ We.T + be, temp, pad mask -75) is folded on the host
  into an 8B/slot bf16 stream.
- Softmax normalization happens after aggregation (denominator constant per
  node); no max-subtraction (scores are O(10)).
- Projection adds a 65th output column = feature-mean of the projected out,
  giving the LN mean for free; variance comes from a y^2 halving tree.
  LN gamma/beta ops are emitted only when non-trivial.
"""

import numpy as np
import ml_dtypes

import concourse.bass as bass
import concourse.bacc as bacc
import concourse.tile as tile
from concourse import mybir
from concourse.bass_utils import run_bass_kernel_spmd
from concourse.masks import make_identity

N, E = 50000, 800000
D_NODE, D_EDGE, H = 64, 32, 4
D_H = D_NODE // H
LN_EPS = 1e-5
NCORES = 8
P = 128
NT = 49                # node tiles per core
NPC = NT * P           # padded nodes per core = 6272
KC = 16                # K|V build slot-columns per PSUM tile
TGMAX = 5              # max tiles per sub-group (same D)
SLOTCAP = 80           # max T*D per sub-group (bounds SBUF slab size)
MASK_VAL = -75.0
F32 = mybir.dt.float32
BF16 = mybir.dt.bfloat16
BF_NP = ml_dtypes.bfloat16


# ---------------------------------------------------------------- host prep --
def _host_prep(node_features, edge_features, edge_index, Wq, bq, Wk, bk, Wv, bv,
               We, be, Wo, bo, ln_gamma, ln_beta, log_temp):
    x = np.ascontiguousarray(np.asarray(node_features, dtype=np.float32))
    ef = np.ascontiguousarray(np.asarray(edge_features, dtype=np.float32))
    src = np.asarray(edge_index[0], dtype=np.int64)
    tgt = np.asarray(edge_index[1], dtype=np.int64)
    temp = np.exp(np.asarray(log_temp, dtype=np.float32))
    gamma = np.asarray(ln_gamma, dtype=np.float32)
    beta = np.asarray(ln_beta, dtype=np.float32)
    bo = np.asarray(bo, dtype=np.float32)

    deg = np.bincount(tgt, minlength=N)
    order = np.argsort(-deg, kind="stable")
    node_lists = []
    for c in range(NCORES):
        nl = order[c::NCORES]
        nl = np.concatenate([nl, np.full(NPC - len(nl), -1, dtype=np.int64)])
        node_lists.append(nl)

    D_t = np.zeros(NT, dtype=np.int64)
    for c in range(NCORES):
        d = np.where(node_lists[c] >= 0, deg[np.maximum(node_lists[c], 0)], 0)
        D_t = np.maximum(D_t, d.reshape(NT, P).max(axis=1))
    D_t = np.maximum(D_t, 2)
    D_t = D_t + (D_t & 1)               # pad to even
    assert D_t.max() <= 128
    SD = int(D_t.sum())

    # sub-groups of equal-D tiles; cap T*D so per-group SBUF slabs stay small
    groups = []                          # (tile_start, T, D)
    t0 = 0
    while t0 < NT:
        D = int(D_t[t0])
        t1 = t0
        while (t1 < NT and int(D_t[t1]) == D and t1 - t0 < TGMAX
               and (t1 - t0 + 1) * D <= SLOTCAP):
            t1 += 1
        groups.append((t0, t1 - t0, D))
        t0 = t1

    eorder = np.argsort(tgt, kind="stable")
    estart = np.zeros(N + 1, dtype=np.int64)
    np.cumsum(deg, out=estart[1:])

    # per-edge bias (ef @ We.T + be) * temp, computed once globally
    ebias = (ef @ np.asarray(We, dtype=np.float32).T
             + np.asarray(be, dtype=np.float32)) * temp[None, :]   # [E, H]

    qscale = (np.repeat(temp, D_H) / np.sqrt(D_H)).astype(np.float32)
    Wq_aug = (np.concatenate([np.asarray(Wq).T, np.asarray(bq)[None, :]], 0)
              * qscale[None, :]).astype(BF_NP)                     # [65,64]
    Wkv_aug = np.concatenate(
        [np.concatenate([np.asarray(Wk).T, np.asarray(Wv).T], 1),
         np.concatenate([np.asarray(bk), np.asarray(bv)])[None, :]], 0
    ).astype(BF_NP)                                                # [65,128]
    # proj weights [64, 65]: col f = Wo[f, :], col 64 = feature-mean of out
    WoT = np.asarray(Wo, dtype=np.float32).T                       # [64c, 64f]
    Wo_proj = np.concatenate(
        [WoT, WoT.mean(axis=1, keepdims=True)], 1).astype(BF_NP)   # [64,65]

    x_aug = np.concatenate(
        [x, np.ones((N, 1), np.float32)], 1).astype(BF_NP)         # [N,65]

    ln_trivial = bool(np.all(gamma == 1.0) and np.all(beta == 0.0))

    per_core = []
    for c in range(NCORES):
        nl = node_lists[c]
        xt = np.zeros((65, SD * P), dtype=BF_NP)
        biasb = np.full((P, SD, H), MASK_VAL, dtype=np.float32)
        doff = 0
        for t in range(NT):
            D = int(D_t[t])
            nlt = nl[t * P:(t + 1) * P]
            degt = np.where(nlt >= 0, deg[np.maximum(nlt, 0)], 0)
            k = np.arange(D)
            valid = k[None, :] < degt[:, None]                    # [P,D]
            pos = estart[np.maximum(nlt, 0)][:, None] + k[None, :]
            eids = eorder[np.minimum(pos, E - 1)]
            eids = np.where(valid, eids, 0)
            gsrc = np.where(valid, src[eids], 0)                  # [P,D]
            # xt column (doff+k)*P + p  <- x_aug[src of slot (p,k)]
            cols = (doff + k[None, :]) * P + np.arange(P)[:, None]
            xa = np.where(valid[:, :, None], x_aug[gsrc], 0)      # [P,D,65]
            xt[:, cols.ravel()] = xa.reshape(P * D, 65).T
            bb = np.where(valid[:, :, None], ebias[eids], MASK_VAL)
            biasb[:, doff:doff + D, :] = bb
            doff += D
        xq = np.where(nl[:, None] >= 0, x[np.maximum(nl, 0)], 0.0)
        xqT_aug = np.concatenate([xq.T, np.ones((1, NPC), np.float32)],
                                 0).astype(BF_NP)                  # [65,NPC]
        # residual (x + bo), transposed per tile: [P, 64, NT]
        xq_r = (xq + bo[None, :]).reshape(NT, P, D_NODE)
        xq_T = np.ascontiguousarray(
            xq_r.transpose(1, 2, 0).astype(BF_NP)).reshape(P, D_NODE * NT)
        mx = np.ascontiguousarray(
            (xq + bo[None, :]).mean(axis=1).reshape(NT, P).T.astype(np.float32))
        per_core.append({
            "xt": xt,
            "biasb": np.ascontiguousarray(
                biasb.reshape(P, SD * H).astype(BF_NP)),
            "xqT": np.ascontiguousarray(xqT_aug),
            "xq_T": xq_T,
            "mx": mx,                                              # [P, NT]
            "wq": np.ascontiguousarray(Wq_aug),
            "wkv": np.ascontiguousarray(Wkv_aug),
            "wo": np.ascontiguousarray(Wo_proj),
            "gb": np.stack([gamma, beta]).astype(np.float32),
        })
    meta = dict(D_seq=[int(d) for d in D_t], groups=groups, SD=SD,
                ln_trivial=ln_trivial)
    return per_core, node_lists, meta


# ------------------------------------------------------------- bass kernel --
def _build_kernel(meta):
    D_seq = meta["D_seq"]
    groups = meta["groups"]
    SD = meta["SD"]
    ln_trivial = meta["ln_trivial"]
    doff_t = np.zeros(NT + 1, dtype=np.int64)
    np.cumsum(np.asarray(D_seq), out=doff_t[1:])

    nc = bacc.Bacc(None, target_bir_lowering=False)

    xt = nc.dram_tensor("xt", [65, SD * P], BF16, kind="ExternalInput")
    biasb = nc.dram_tensor("biasb", [P, SD * H], BF16, kind="ExternalInput")
    xqT = nc.dram_tensor("xqT", [65, NPC], BF16, kind="ExternalInput")
    xq_T = nc.dram_tensor("xq_T", [P, D_NODE * NT], BF16, kind="ExternalInput")
    mx = nc.dram_tensor("mx", [P, NT], F32, kind="ExternalInput")
    wq = nc.dram_tensor("wq", [65, D_NODE], BF16, kind="ExternalInput")
    wkv = nc.dram_tensor("wkv", [65, 2 * D_NODE], BF16, kind="ExternalInput")
    wo = nc.dram_tensor("wo", [D_NODE, 65], BF16, kind="ExternalInput")
    gb = nc.dram_tensor("gb", [2, D_NODE], F32, kind="ExternalInput")
    y = nc.dram_tensor("y", [P, D_NODE * NT], BF16, kind="ExternalOutput")

    with tile.TileContext(nc) as tc, nc.allow_low_precision(reason="bf16 kernel"):
        with (
            tc.tile_pool(name="singles", bufs=1) as singles,
        ):
            wq_sb = singles.tile([65, D_NODE], BF16)
            nc.sync.dma_start(out=wq_sb[:], in_=wq[:])
            wkv_sb = singles.tile([65, 2 * D_NODE], BF16)
            nc.sync.dma_start(out=wkv_sb[:], in_=wkv[:])
            wo_sb = singles.tile([D_NODE, 65], BF16)
            nc.sync.dma_start(out=wo_sb[:], in_=wo[:])
            xqT_sb = singles.tile([65, NPC], BF16)
            nc.sync.dma_start(out=xqT_sb[:], in_=xqT[:])
            xqr_sb = singles.tile([P, D_NODE, NT], BF16)
            nc.sync.dma_start(out=xqr_sb[:].rearrange("p f t -> p (f t)"),
                              in_=xq_T[:])
            mx_sb = singles.tile([P, NT], F32)
            nc.sync.dma_start(out=mx_sb[:], in_=mx[:])
            bias_sb = singles.tile([P, SD, H], BF16)
            nc.sync.dma_start(out=bias_sb[:].rearrange("p s h -> p (s h)"),
                              in_=biasb[:])
            ident = singles.tile([P, P], BF16)
            make_identity(nc, ident[:])
            eps_sb = singles.tile([P, 1], F32)
            nc.vector.memset(eps_sb[:], LN_EPS)
            tiny_sb = singles.tile([P, 1], F32)
            nc.vector.memset(tiny_sb[:], 1e-10)

            q_all = singles.tile([P, NT, D_NODE], BF16)
            unn_all = singles.tile([P, NT, D_NODE], BF16)
            outn_all = singles.tile([P, NT, D_NODE], BF16)
            yt_sb = singles.tile([P, D_NODE, NT], BF16)
            mu_sb = singles.tile([P, NT], F32)
            var_sb = singles.tile([P, NT], F32)

            # ---------------- Q phase: q_all = (x @ Wq')  ----------------
            with (
                tc.tile_pool(name="pq", bufs=2, space="PSUM") as pq,
            ):
                for t0 in range(0, NT, 8):
                    tn = min(8, NT - t0)
                    qp = pq.tile([P, 8, D_NODE], F32, tag="qp", name="qp")
                    for j in range(tn):
                        t = t0 + j
                        nc.tensor.matmul(out=qp[:, j, :],
                                         lhsT=xqT_sb[:, t * P:(t + 1) * P],
                                         rhs=wq_sb[:], start=True, stop=True)
                    nc.scalar.copy(out=q_all[:, t0:t0 + tn, :],
                                   in_=qp[:, 0:tn, :])

            # ---------------- main loop over sub-groups ----------------
            with (
                tc.tile_pool(name="xtp", bufs=3) as xtp,
                tc.tile_pool(name="kvp", bufs=2, space="PSUM") as kvp,
                tc.tile_pool(name="sg", bufs=2) as sgp,
                tc.tile_pool(name="sml", bufs=2) as smlp,
            ):
                for (ts, T, D) in groups:
                    qkp = sgp.tile([P, T, D, D_NODE], BF16, tag="qkp",
                                   name="qkp")
                    v_t = sgp.tile([P, T, D_NODE, D], BF16, tag="v_t",
                                   name="v_t")
                    for tt in range(T):
                        t = ts + tt
                        dof = int(doff_t[t])
                        xt_sb = xtp.tile([65, D * P], BF16, tag="xt",
                                         name="xt_sb")
                        nc.sync.dma_start(
                            out=xt_sb[:],
                            in_=xt[:, dof * P:(dof + D) * P])
                        for kc in range(0, D, KC):
                            kn = min(KC, D - kc)
                            pt = kvp.tile([P, KC, 2 * D_NODE], F32, tag="pt",
                                          name="pt")
                            for j in range(kn):
                                nc.tensor.matmul(
                                    out=pt[:, j, :],
                                    lhsT=xt_sb[:, (kc + j) * P:(kc + j + 1) * P],
                                    rhs=wkv_sb[:], start=True, stop=True)
                            # qk mul on Pool straight from PSUM
                            qsl = q_all[:, t, :]
                            q_b = bass.AP(
                                tensor=qsl.tensor, offset=qsl.offset,
                                ap=[qsl.ap[0], [0, kn], [1, D_NODE]])
                            nc.gpsimd.tensor_mul(
                                out=qkp[:, tt, kc:kc + kn, :],
                                in0=pt[:, 0:kn, 0:D_NODE], in1=q_b)
                            # V copy (feature-major) on ACT
                            vdst = v_t[:, tt, :, kc:kc + kn]
                            nc.scalar.copy(
                                out=vdst.rearrange("p f k -> p k f"),
                                in_=pt[:, 0:kn, D_NODE:2 * D_NODE])

                    # ---- score d-reduction: halving tree over 16 ----
                    qv = qkp[:].rearrange("p t d (h w) -> p (t d h) w", h=H)
                    s1 = sgp.tile([P, T * D * H, 8], BF16, tag="s1", name="s1")
                    nc.vector.tensor_add(out=s1[:], in0=qv[:, :, 0:8],
                                         in1=qv[:, :, 8:16])
                    s2 = sgp.tile([P, T * D * H, 4], BF16, tag="s2", name="s2")
                    nc.vector.tensor_add(out=s2[:], in0=s1[:, :, 0:4],
                                         in1=s1[:, :, 4:8])
                    s3 = sgp.tile([P, T * D * H, 2], BF16, tag="s3", name="s3")
                    nc.vector.tensor_add(out=s3[:], in0=s2[:, :, 0:2],
                                         in1=s2[:, :, 2:4])
                    sc = sgp.tile([P, T, D, H], BF16, tag="sc", name="sc")
                    scv = sc[:].rearrange("p t d h -> p (t d h)")
                    nc.vector.tensor_add(
                        out=scv,
                        in0=bass.AP(tensor=s3[:].tensor, offset=s3[:].offset,
                                    ap=[s3[:].ap[0], [2, T * D * H]]),
                        in1=bass.AP(tensor=s3[:].tensor,
                                    offset=s3[:].offset + 1,
                                    ap=[s3[:].ap[0], [2, T * D * H]]))
                    sc2 = sgp.tile([P, T, D, H], BF16, tag="sc2", name="sc2")
                    dos = int(doff_t[ts])
                    nc.vector.tensor_add(out=sc2[:], in0=sc[:],
                                         in1=bias_sb[:, dos:dos + T * D, :]
                                         .rearrange("p (t d) h -> p t d h", t=T))
                    # exp -> ex_t [P, T, H, D] (h-major for den + exv bcast)
                    ex_t = sgp.tile([P, T, H, D], BF16, tag="ex", name="ex_t")
                    exd = ex_t[:].rearrange("p t h d -> p t d h")
                    nc.scalar.activation(out=exd, in_=sc2[:],
                                         func=mybir.ActivationFunctionType.Exp)

                    # ---- denominator: halving tree over D ----
                    exm = ex_t[:].rearrange("p t h d -> p (t h) d")
                    hw_ = D // 2
                    d1 = smlp.tile([P, T * H, hw_], BF16, tag="d1", name="d1")
                    nc.vector.tensor_add(out=d1[:], in0=exm[:, :, 0:hw_],
                                         in1=exm[:, :, hw_:2 * hw_])
                    den = smlp.tile([P, T, H], F32, tag="den", name="den")
                    dv = den[:].rearrange("p t h -> p (t h)")
                    if hw_ % 2 == 0 and hw_ > 2:
                        qw = hw_ // 2
                        d2 = smlp.tile([P, T * H, qw], BF16, tag="d2",
                                       name="d2")
                        nc.vector.tensor_add(out=d2[:], in0=d1[:, :, 0:qw],
                                             in1=d1[:, :, qw:2 * qw])
                        nc.vector.tensor_reduce(
                            out=dv, in_=d2[:], axis=mybir.AxisListType.X,
                            op=mybir.AluOpType.add)
                    else:
                        nc.vector.tensor_reduce(
                            out=dv, in_=d1[:], axis=mybir.AxisListType.X,
                            op=mybir.AluOpType.add)
                    rden = smlp.tile([P, T, H], BF16, tag="rden", name="rden")
                    nc.vector.reciprocal(out=rden[:], in_=den[:])

                    # ---- exv = V_t * ex (bcast over d_h) ----
                    exv = sgp.tile([P, T, D_NODE, D], BF16, tag="exv",
                                   name="exv")
                    vv = v_t[:].rearrange("p t (h w) d -> p (t h) w d", h=H)
                    ev = exv[:].rearrange("p t (h w) d -> p (t h) w d", h=H)
                    exb = bass.AP(
                        tensor=ex_t[:].tensor, offset=ex_t[:].offset,
                        ap=[ex_t[:].ap[0], [D, T * H], [0, D_H], [1, D]])
                    nc.vector.tensor_mul(out=ev, in0=vv, in1=exb)

                    # ---- unn: halving tree over D ----
                    evm = exv[:].rearrange("p t f d -> p (t f) d")
                    u1 = sgp.tile([P, T * D_NODE, hw_], BF16, tag="u1",
                                  name="u1")
                    nc.vector.tensor_add(out=u1[:], in0=evm[:, :, 0:hw_],
                                         in1=evm[:, :, hw_:2 * hw_])
                    udst = unn_all[:, ts:ts + T, :].rearrange(
                        "p t f -> p (t f)")
                    if hw_ % 2 == 0 and hw_ > 2:
                        qw = hw_ // 2
                        u2 = sgp.tile([P, T * D_NODE, qw], BF16, tag="u2",
                                      name="u2")
                        nc.vector.tensor_add(out=u2[:], in0=u1[:, :, 0:qw],
                                             in1=u1[:, :, qw:2 * qw])
                        nc.vector.tensor_reduce(
                            out=udst, in_=u2[:], axis=mybir.AxisListType.X,
                            op=mybir.AluOpType.add)
                    else:
                        nc.vector.tensor_reduce(
                            out=udst, in_=u1[:], axis=mybir.AxisListType.X,
                            op=mybir.AluOpType.add)

                    # ---- outn = unn * rden ----
                    rdb = bass.AP(
                        tensor=rden[:].tensor, offset=rden[:].offset,
                        ap=[rden[:].ap[0], [1, T * H], [0, D_H]])
                    nc.vector.tensor_mul(
                        out=outn_all[:, ts:ts + T, :].rearrange(
                            "p t (h w) -> p (t h) w", h=H),
                        in0=unn_all[:, ts:ts + T, :].rearrange(
                            "p t (h w) -> p (t h) w", h=H),
                        in1=rdb)

            # ---------------- projection + residual + mu ----------------
            with (
                tc.tile_pool(name="ptp", bufs=2, space="PSUM") as ptp,
                tc.tile_pool(name="pyp", bufs=2, space="PSUM") as pyp,
                tc.tile_pool(name="tps", bufs=2) as tpsp,
            ):
                for t0 in range(0, NT, 4):
                    tn = min(4, NT - t0)
                    tp = ptp.tile([D_NODE, 4, P], BF16, tag="tp", name="tp")
                    for j in range(tn):
                        nc.tensor.transpose(
                            out=tp[:, j, :], in_=outn_all[:, t0 + j, :],
                            identity=ident[:])
                    tps = tpsp.tile([D_NODE, 4, P], BF16, tag="tps",
                                    name="tps")
                    nc.scalar.copy(out=tps[:, 0:tn, :], in_=tp[:, 0:tn, :])
                    yp = pyp.tile([P, 4, 65], F32, tag="yp", name="yp")
                    for j in range(tn):
                        nc.tensor.matmul(out=yp[:, j, :], lhsT=tps[:, j, :],
                                         rhs=wo_sb[:], start=True, stop=True)
                    # residual into y_T (transposed) layout
                    nc.vector.tensor_add(
                        out=yt_sb[:, :, t0:t0 + tn],
                        in0=yp[:, 0:tn, 0:D_NODE].rearrange(
                            "p t f -> p f t"),
                        in1=xqr_sb[:, :, t0:t0 + tn])
                    # mu = mx + mean(out)
                    nc.vector.tensor_add(
                        out=mu_sb[:, t0:t0 + tn],
                        in0=yp[:, 0:tn, 64:65].rearrange("p t o -> p (t o)"),
                        in1=mx_sb[:, t0:t0 + tn])

            # ---------------- layernorm epilogue ----------------
            with (
                tc.tile_pool(name="lnp", bufs=1) as lnp,
            ):
                ysq = lnp.tile([P, D_NODE, NT], BF16, name="ysq")
                nc.gpsimd.tensor_mul(out=ysq[:], in0=yt_sb[:], in1=yt_sb[:])
                w = D_NODE // 2
                cur = ysq
                while w >= 1:
                    nxt = lnp.tile([P, w, NT], BF16, name=f"vs{w}")
                    nc.vector.tensor_add(out=nxt[:], in0=cur[:, 0:w, :],
                                         in1=cur[:, w:2 * w, :])
                    cur = nxt
                    w //= 2
                # var = sumsq/64 - mu^2
                ss = lnp.tile([P, NT], F32, name="ss")
                nc.vector.tensor_scalar_mul(
                    ss[:], cur[:].rearrange("p o t -> p (o t)"), 1.0 / D_NODE)
                musq = lnp.tile([P, NT], F32, name="musq")
                nc.vector.tensor_mul(out=musq[:], in0=mu_sb[:], in1=mu_sb[:])
                nc.vector.tensor_sub(out=var_sb[:], in0=ss[:], in1=musq[:])
                sd = lnp.tile([P, NT], F32, name="sd")
                nc.scalar.activation(out=sd[:], in_=var_sb[:],
                                     func=mybir.ActivationFunctionType.Sqrt,
                                     bias=eps_sb[:])
                rstd = lnp.tile([P, NT], BF16, name="rstd")
                nc.vector.reciprocal(out=rstd[:], in_=sd[:])
                musd = lnp.tile([P, NT], BF16, name="musd")
                nc.vector.tensor_mul(out=musd[:], in0=mu_sb[:], in1=rstd[:])
                n1 = lnp.tile([P, D_NODE, NT], BF16, name="n1")
                rb = bass.AP(tensor=rstd[:].tensor, offset=rstd[:].offset,
                             ap=[rstd[:].ap[0], [0, D_NODE], [1, NT]])
                nc.vector.tensor_mul(out=n1[:], in0=yt_sb[:], in1=rb)
                mb = bass.AP(tensor=musd[:].tensor, offset=musd[:].offset,
                             ap=[musd[:].ap[0], [0, D_NODE], [1, NT]])
                yout = lnp.tile([P, D_NODE, NT], BF16, name="yout")
                nc.vector.tensor_sub(out=yout[:], in0=n1[:], in1=mb)
                if not ln_trivial:
                    gam = lnp.tile([P, D_NODE], F32, name="gam")
                    nc.sync.dma_start(
                        out=gam[:],
                        in_=bass.AP(tensor=gb[:].tensor, offset=0,
                                    ap=[[0, P], [1, D_NODE]]))
                    bet = lnp.tile([P, D_NODE], F32, name="bet")
                    nc.sync.dma_start(
                        out=bet[:],
                        in_=bass.AP(tensor=gb[:].tensor, offset=D_NODE,
                                    ap=[[0, P], [1, D_NODE]]))
                    gbc = bass.AP(tensor=gam[:].tensor, offset=gam[:].offset,
                                  ap=[gam[:].ap[0], [1, D_NODE], [0, NT]])
                    bbc = bass.AP(tensor=bet[:].tensor, offset=bet[:].offset,
                                  ap=[bet[:].ap[0], [1, D_NODE], [0, NT]])
                    nc.vector.tensor_mul(out=yout[:], in0=yout[:], in1=gbc)
                    nc.vector.tensor_add(out=yout[:], in0=yout[:], in1=bbc)
                nc.sync.dma_start(
                    out=y[:], in_=yout[:].rearrange("p f t -> p (f t)"))

    nc.compile()
    return nc


# ------------------------------------------------------------------ driver --
def kernel(**inputs) -> np.ndarray:
    per_core, node_lists, meta = _host_prep(**inputs)
    nc = _build_kernel(meta)
    res = run_bass_kernel_spmd(nc, per_core, core_ids=list(range(NCORES)))
    y_full = np.zeros((N, D_NODE), dtype=np.float32)
    for c in range(NCORES):
        yc = res.results[c]["y"].astype(np.float32)
        yc = yc.reshape(P, D_NODE, NT).transpose(2, 0, 1).reshape(NPC, D_NODE)
        nl = node_lists[c]
        real = nl >= 0
        y_full[nl[real]] = yc[real]
    return y_full


# revision 17
# speedup vs baseline: 1.9336x; 1.0355x over previous
"""NodeAttention (GNN scatter-softmax attention) on 8 Trainium2 NeuronCores.

Strategy (v2 — no DRAM KV roundtrip):
- Host deals nodes to 8 cores round-robin by degree rank; one static NEFF
  serves all cores (SPMD).  Per core: 49 node-tiles x 128 nodes; tile t gets a
  dense slot grid [128, D_t] (D_t = max degree in tile across cores, padded to
  even).
- The host replicates x per SLOT in contraction-major order (xt stream); the
  device builds K|V for 16 slot-columns at a time straight into PSUM via
  matmuls.  Pool multiplies Q against the K half directly from PSUM (no K
  copy); ACT copies the V half to SBUF feature-major.
- Score d-reduction and the V k-aggregation use contiguous halving-add trees
  on DVE (2x bf16 mode) instead of 1x tensor_reduce.
- Per-edge bias (ef @ We.T + be, temp, pad mask -75) is folded on the host
  into an 8B/slot bf16 stream.
- Softmax normalization happens after aggregation (denominator constant per
  node); no max-subtraction (scores are O(10)).
- Projection adds a 65th output column = feature-mean of the projected out,
  giving the LN mean for free; variance comes from a y^2 halving tree.
  LN gamma/beta ops are emitted only when non-trivial.
"""

import numpy as np
import ml_dtypes

import concourse.bass as bass
import concourse.bacc as bacc
import concourse.tile as tile
from concourse import mybir
from concourse.bass_utils import run_bass_kernel_spmd
from concourse.masks import make_identity

N, E = 50000, 800000
D_NODE, D_EDGE, H = 64, 32, 4
D_H = D_NODE // H
LN_EPS = 1e-5
NCORES = 8
P = 128
NT = 49                # node tiles per core
NPC = NT * P           # padded nodes per core = 6272
KC = 16                # K|V build slot-columns per PSUM tile
TGMAX = 6              # max tiles per sub-group (same D)
SLOTCAP = 96           # max T*D per sub-group (bounds SBUF slab size)
MASK_VAL = -75.0
F32 = mybir.dt.float32
BF16 = mybir.dt.bfloat16
BF_NP = ml_dtypes.bfloat16


# ---------------------------------------------------------------- host prep --
def _host_prep(node_features, edge_features, edge_index, Wq, bq, Wk, bk, Wv, bv,
               We, be, Wo, bo, ln_gamma, ln_beta, log_temp):
    x = np.ascontiguousarray(np.asarray(node_features, dtype=np.float32))
    ef = np.ascontiguousarray(np.asarray(edge_features, dtype=np.float32))
    src = np.asarray(edge_index[0], dtype=np.int64)
    tgt = np.asarray(edge_index[1], dtype=np.int64)
    temp = np.exp(np.asarray(log_temp, dtype=np.float32))
    gamma = np.asarray(ln_gamma, dtype=np.float32)
    beta = np.asarray(ln_beta, dtype=np.float32)
    bo = np.asarray(bo, dtype=np.float32)

    deg = np.bincount(tgt, minlength=N)
    order = np.argsort(-deg, kind="stable")
    node_lists = []
    for c in range(NCORES):
        nl = order[c::NCORES]
        nl = np.concatenate([nl, np.full(NPC - len(nl), -1, dtype=np.int64)])
        node_lists.append(nl)

    D_t = np.zeros(NT, dtype=np.int64)
    for c in range(NCORES):
        d = np.where(node_lists[c] >= 0, deg[np.maximum(node_lists[c], 0)], 0)
        D_t = np.maximum(D_t, d.reshape(NT, P).max(axis=1))
    D_t = np.maximum(D_t, 2)
    D_t = D_t + (D_t & 1)               # pad to even
    assert D_t.max() <= 128
    SD = int(D_t.sum())

    # sub-groups of equal-D tiles; cap T*D so per-group SBUF slabs stay small
    groups = []                          # (tile_start, T, D)
    t0 = 0
    while t0 < NT:
        D = int(D_t[t0])
        t1 = t0
        while (t1 < NT and int(D_t[t1]) == D and t1 - t0 < TGMAX
               and (t1 - t0 + 1) * D <= SLOTCAP):
            t1 += 1
        groups.append((t0, t1 - t0, D))
        t0 = t1

    eorder = np.argsort(tgt, kind="stable")
    estart = np.zeros(N + 1, dtype=np.int64)
    np.cumsum(deg, out=estart[1:])

    # per-edge bias (ef @ We.T + be) * temp, computed once globally
    ebias = (ef @ np.asarray(We, dtype=np.float32).T
             + np.asarray(be, dtype=np.float32)) * temp[None, :]   # [E, H]

    qscale = (np.repeat(temp, D_H) / np.sqrt(D_H)).astype(np.float32)
    Wq_aug = (np.concatenate([np.asarray(Wq).T, np.asarray(bq)[None, :]], 0)
              * qscale[None, :]).astype(BF_NP)                     # [65,64]
    Wkv_aug = np.concatenate(
        [np.concatenate([np.asarray(Wk).T, np.asarray(Wv).T], 1),
         np.concatenate([np.asarray(bk), np.asarray(bv)])[None, :]], 0
    ).astype(BF_NP)                                                # [65,128]
    # proj weights [64, 65]: col f = Wo[f, :], col 64 = feature-mean of out
    WoT = np.asarray(Wo, dtype=np.float32).T                       # [64c, 64f]
    Wo_proj = np.concatenate(
        [WoT, WoT.mean(axis=1, keepdims=True)], 1).astype(BF_NP)   # [64,65]

    x_aug = np.concatenate(
        [x, np.ones((N, 1), np.float32)], 1).astype(BF_NP)         # [N,65]

    ln_trivial = bool(np.all(gamma == 1.0) and np.all(beta == 0.0))

    per_core = []
    for c in range(NCORES):
        nl = node_lists[c]
        xt = np.zeros((65, SD * P), dtype=BF_NP)
        biasb = np.full((P, SD, H), MASK_VAL, dtype=np.float32)
        doff = 0
        for t in range(NT):
            D = int(D_t[t])
            nlt = nl[t * P:(t + 1) * P]
            degt = np.where(nlt >= 0, deg[np.maximum(nlt, 0)], 0)
            k = np.arange(D)
            valid = k[None, :] < degt[:, None]                    # [P,D]
            pos = estart[np.maximum(nlt, 0)][:, None] + k[None, :]
            eids = eorder[np.minimum(pos, E - 1)]
            eids = np.where(valid, eids, 0)
            gsrc = np.where(valid, src[eids], 0)                  # [P,D]
            # xt column (doff+k)*P + p  <- x_aug[src of slot (p,k)]
            cols = (doff + k[None, :]) * P + np.arange(P)[:, None]
            xa = np.where(valid[:, :, None], x_aug[gsrc], 0)      # [P,D,65]
            xt[:, cols.ravel()] = xa.reshape(P * D, 65).T
            bb = np.where(valid[:, :, None], ebias[eids], MASK_VAL)
            biasb[:, doff:doff + D, :] = bb
            doff += D
        xq = np.where(nl[:, None] >= 0, x[np.maximum(nl, 0)], 0.0)
        xqT_aug = np.concatenate([xq.T, np.ones((1, NPC), np.float32)],
                                 0).astype(BF_NP)                  # [65,NPC]
        # residual (x + bo), transposed per tile: [P, 64, NT]
        xq_r = (xq + bo[None, :]).reshape(NT, P, D_NODE)
        xq_T = np.ascontiguousarray(
            xq_r.transpose(1, 2, 0).astype(BF_NP)).reshape(P, D_NODE * NT)
        mx = np.ascontiguousarray(
            (xq + bo[None, :]).mean(axis=1).reshape(NT, P).T.astype(np.float32))
        per_core.append({
            "xt": xt,
            "biasb": np.ascontiguousarray(
                biasb.reshape(P, SD * H).astype(BF_NP)),
            "xqT": np.ascontiguousarray(xqT_aug),
            "xq_T": xq_T,
            "mx": mx,                                              # [P, NT]
            "wq": np.ascontiguousarray(Wq_aug),
            "wkv": np.ascontiguousarray(Wkv_aug),
            "wo": np.ascontiguousarray(Wo_proj),
            "gb": np.stack([gamma, beta]).astype(np.float32),
        })
    meta = dict(D_seq=[int(d) for d in D_t], groups=groups, SD=SD,
                ln_trivial=ln_trivial)
    return per_core, node_lists, meta


# ------------------------------------------------------------- bass kernel --
def _build_kernel(meta):
    D_seq = meta["D_seq"]
    groups = meta["groups"]
    SD = meta["SD"]
    ln_trivial = meta["ln_trivial"]
    doff_t = np.zeros(NT + 1, dtype=np.int64)
    np.cumsum(np.asarray(D_seq), out=doff_t[1:])

    nc = bacc.Bacc(None, target_bir_lowering=False)

    xt = nc.dram_tensor("xt", [65, SD * P], BF16, kind="ExternalInput")
    biasb = nc.dram_tensor("biasb", [P, SD * H], BF16, kind="ExternalInput")
    xqT = nc.dram_tensor("xqT", [65, NPC], BF16, kind="ExternalInput")
    xq_T = nc.dram_tensor("xq_T", [P, D_NODE * NT], BF16, kind="ExternalInput")
    mx = nc.dram_tensor("mx", [P, NT], F32, kind="ExternalInput")
    wq = nc.dram_tensor("wq", [65, D_NODE], BF16, kind="ExternalInput")
    wkv = nc.dram_tensor("wkv", [65, 2 * D_NODE], BF16, kind="ExternalInput")
    wo = nc.dram_tensor("wo", [D_NODE, 65], BF16, kind="ExternalInput")
    gb = nc.dram_tensor("gb", [2, D_NODE], F32, kind="ExternalInput")
    y = nc.dram_tensor("y", [P, D_NODE * NT], BF16, kind="ExternalOutput")

    with tile.TileContext(nc) as tc, nc.allow_low_precision(reason="bf16 kernel"):
        with (
            tc.tile_pool(name="singles", bufs=1) as singles,
        ):
            wq_sb = singles.tile([65, D_NODE], BF16)
            nc.sync.dma_start(out=wq_sb[:], in_=wq[:])
            wkv_sb = singles.tile([65, 2 * D_NODE], BF16)
            nc.sync.dma_start(out=wkv_sb[:], in_=wkv[:])
            wo_sb = singles.tile([D_NODE, 65], BF16)
            nc.scalar.dma_start(out=wo_sb[:], in_=wo[:])
            xqT_sb = singles.tile([65, NPC], BF16)
            xqr_sb = singles.tile([P, D_NODE, NT], BF16)
            nc.gpsimd.dma_start(out=xqr_sb[:].rearrange("p f t -> p (f t)"),
                                in_=xq_T[:])
            mx_sb = singles.tile([P, NT], F32)
            nc.gpsimd.dma_start(out=mx_sb[:], in_=mx[:])
            bias_sb = singles.tile([P, SD, H], BF16)
            nc.scalar.dma_start(out=bias_sb[:].rearrange("p s h -> p (s h)"),
                                in_=biasb[:])
            ident = singles.tile([P, P], BF16)
            make_identity(nc, ident[:])
            eps_sb = singles.tile([P, 1], F32)
            nc.vector.memset(eps_sb[:], LN_EPS)
            tiny_sb = singles.tile([P, 1], F32)
            nc.vector.memset(tiny_sb[:], 1e-10)

            q_all = singles.tile([P, NT, D_NODE], BF16)
            unn_all = singles.tile([P, NT, D_NODE], BF16)
            outn_all = singles.tile([P, NT, D_NODE], BF16)
            yt_sb = singles.tile([P, D_NODE, NT], BF16)
            mu_sb = singles.tile([P, NT], F32)
            var_sb = singles.tile([P, NT], F32)

            # ---------------- Q phase: q_all = (x @ Wq')  ----------------
            with (
                tc.tile_pool(name="pq", bufs=2, space="PSUM") as pq,
            ):
                for t0 in range(0, NT, 8):
                    tn = min(8, NT - t0)
                    nc.scalar.dma_start(
                        out=xqT_sb[:, t0 * P:(t0 + tn) * P],
                        in_=xqT[:, t0 * P:(t0 + tn) * P])
                    qp = pq.tile([P, 8, D_NODE], F32, tag="qp", name="qp")
                    for j in range(tn):
                        t = t0 + j
                        nc.tensor.matmul(out=qp[:, j, :],
                                         lhsT=xqT_sb[:, t * P:(t + 1) * P],
                                         rhs=wq_sb[:], start=True, stop=True)
                    nc.scalar.copy(out=q_all[:, t0:t0 + tn, :],
                                   in_=qp[:, 0:tn, :])

            # ---------------- main loop over sub-groups ----------------
            with (
                tc.tile_pool(name="xtp", bufs=3) as xtp,
                tc.tile_pool(name="kvp", bufs=2, space="PSUM") as kvp,
                tc.tile_pool(name="sg", bufs=2) as sgp,
                tc.tile_pool(name="sml", bufs=2) as smlp,
            ):
                def halving_tree(src, R, w, tag, pool_, dst, first_eng=None):
                    """Reduce [P, R, w] bf16 view `src` along w into dense
                    [P, R] view `dst` via halving adds (2x mode) while the
                    width is even; odd tail uses one 1x tensor_reduce."""
                    cur = src
                    lvl = 0
                    while True:
                        if w % 2:
                            nc.vector.tensor_reduce(
                                out=dst, in_=cur, axis=mybir.AxisListType.X,
                                op=mybir.AluOpType.add)
                            return
                        nw = w // 2
                        eng = first_eng if (lvl == 0 and first_eng) else nc.vector
                        if nw == 1:
                            in0 = bass.AP(tensor=cur.tensor, offset=cur.offset,
                                          ap=[cur.ap[0], [2, R]])
                            in1 = bass.AP(tensor=cur.tensor,
                                          offset=cur.offset + 1,
                                          ap=[cur.ap[0], [2, R]])
                            eng.tensor_add(out=dst, in0=in0, in1=in1)
                            return
                        t_ = pool_.tile([P, R, nw], BF16, tag=f"{tag}{lvl}",
                                        name=f"{tag}{lvl}")
                        eng.tensor_add(out=t_[:], in0=cur[:, :, 0:nw],
                                       in1=cur[:, :, nw:2 * nw])
                        cur = t_[:]
                        w = nw
                        lvl += 1

                for (ts, T, D) in groups:
                    qkp = sgp.tile([P, T, D, D_NODE], BF16, tag="qkp",
                                   name="qkp")
                    v_t = sgp.tile([P, T, D_NODE, D], BF16, tag="v_t",
                                   name="v_t")
                    for tt in range(T):
                        t = ts + tt
                        dof = int(doff_t[t])
                        xt_sb = xtp.tile([65, D * P], BF16, tag="xt",
                                         name="xt_sb")
                        nc.sync.dma_start(
                            out=xt_sb[:],
                            in_=xt[:, dof * P:(dof + D) * P])
                        for kc in range(0, D, KC):
                            kn = min(KC, D - kc)
                            pt = kvp.tile([P, KC, 2 * D_NODE], F32, tag="pt",
                                          name="pt")
                            for j in range(kn):
                                nc.tensor.matmul(
                                    out=pt[:, j, :],
                                    lhsT=xt_sb[:, (kc + j) * P:(kc + j + 1) * P],
                                    rhs=wkv_sb[:], start=True, stop=True)
                            # qk mul on Pool straight from PSUM
                            qsl = q_all[:, t, :]
                            q_b = bass.AP(
                                tensor=qsl.tensor, offset=qsl.offset,
                                ap=[qsl.ap[0], [0, kn], [1, D_NODE]])
                            nc.gpsimd.tensor_mul(
                                out=qkp[:, tt, kc:kc + kn, :],
                                in0=pt[:, 0:kn, 0:D_NODE], in1=q_b)
                            # V copy (feature-major) on ACT
                            vdst = v_t[:, tt, :, kc:kc + kn]
                            nc.scalar.copy(
                                out=vdst.rearrange("p f k -> p k f"),
                                in_=pt[:, 0:kn, D_NODE:2 * D_NODE])

                    # ---- score d-reduction: halving tree over 16 ----
                    qv = qkp[:].rearrange("p t d (h w) -> p (t d h) w", h=H)
                    sc = sgp.tile([P, T, D, H], BF16, tag="sc", name="sc")
                    halving_tree(qv, T * D * H, D_H, "s", sgp,
                                 sc[:].rearrange("p t d h -> p (t d h)"))
                    sc2 = sgp.tile([P, T, D, H], BF16, tag="sc2", name="sc2")
                    dos = int(doff_t[ts])
                    nc.vector.tensor_add(out=sc2[:], in0=sc[:],
                                         in1=bias_sb[:, dos:dos + T * D, :]
                                         .rearrange("p (t d) h -> p t d h", t=T))
                    # exp -> ex_t [P, T, H, D] (h-major for den + exv bcast)
                    ex_t = sgp.tile([P, T, H, D], BF16, tag="ex", name="ex_t")
                    exd = ex_t[:].rearrange("p t h d -> p t d h")
                    nc.scalar.activation(out=exd, in_=sc2[:],
                                         func=mybir.ActivationFunctionType.Exp)

                    # ---- denominator: halving tree over D ----
                    exm = ex_t[:].rearrange("p t h d -> p (t h) d")
                    den = smlp.tile([P, T, H], BF16, tag="den", name="den")
                    halving_tree(exm, T * H, D, "d", smlp,
                                 den[:].rearrange("p t h -> p (t h)"))
                    rden = smlp.tile([P, T, H], BF16, tag="rden", name="rden")
                    nc.vector.reciprocal(out=rden[:], in_=den[:])

                    # ---- exv = V_t * ex (bcast over d_h) ----
                    exv = sgp.tile([P, T, D_NODE, D], BF16, tag="exv",
                                   name="exv")
                    vv = v_t[:].rearrange("p t (h w) d -> p (t h) w d", h=H)
                    ev = exv[:].rearrange("p t (h w) d -> p (t h) w d", h=H)
                    exb = bass.AP(
                        tensor=ex_t[:].tensor, offset=ex_t[:].offset,
                        ap=[ex_t[:].ap[0], [D, T * H], [0, D_H], [1, D]])
                    nc.vector.tensor_mul(out=ev, in0=vv, in1=exb)

                    # ---- unn: halving tree over D (L1 on Pool) ----
                    evm = exv[:].rearrange("p t f d -> p (t f) d")
                    udst = unn_all[:, ts:ts + T, :].rearrange(
                        "p t f -> p (t f)")
                    halving_tree(evm, T * D_NODE, D, "u", sgp, udst,
                                 first_eng=nc.gpsimd)

                    # ---- outn = unn * rden ----
                    rdb = bass.AP(
                        tensor=rden[:].tensor, offset=rden[:].offset,
                        ap=[rden[:].ap[0], [1, T * H], [0, D_H]])
                    nc.vector.tensor_mul(
                        out=outn_all[:, ts:ts + T, :].rearrange(
                            "p t (h w) -> p (t h) w", h=H),
                        in0=unn_all[:, ts:ts + T, :].rearrange(
                            "p t (h w) -> p (t h) w", h=H),
                        in1=rdb)

            # ---------------- projection + residual + mu ----------------
            with (
                tc.tile_pool(name="ptp", bufs=2, space="PSUM") as ptp,
                tc.tile_pool(name="pyp", bufs=2, space="PSUM") as pyp,
                tc.tile_pool(name="tps", bufs=2) as tpsp,
            ):
                for t0 in range(0, NT, 4):
                    tn = min(4, NT - t0)
                    tp = ptp.tile([D_NODE, 4, P], BF16, tag="tp", name="tp")
                    for j in range(tn):
                        nc.tensor.transpose(
                            out=tp[:, j, :], in_=outn_all[:, t0 + j, :],
                            identity=ident[:])
                    tps = tpsp.tile([D_NODE, 4, P], BF16, tag="tps",
                                    name="tps")
                    nc.scalar.copy(out=tps[:, 0:tn, :], in_=tp[:, 0:tn, :])
                    yp = pyp.tile([P, 4, 65], F32, tag="yp", name="yp")
                    for j in range(tn):
                        nc.tensor.matmul(out=yp[:, j, :], lhsT=tps[:, j, :],
                                         rhs=wo_sb[:], start=True, stop=True)
                    # residual into y_T (transposed) layout
                    nc.vector.tensor_add(
                        out=yt_sb[:, :, t0:t0 + tn],
                        in0=yp[:, 0:tn, 0:D_NODE].rearrange(
                            "p t f -> p f t"),
                        in1=xqr_sb[:, :, t0:t0 + tn])
                    # mu = mx + mean(out)
                    nc.vector.tensor_add(
                        out=mu_sb[:, t0:t0 + tn],
                        in0=yp[:, 0:tn, 64:65].rearrange("p t o -> p (t o)"),
                        in1=mx_sb[:, t0:t0 + tn])

            # ---------------- layernorm epilogue ----------------
            with (
                tc.tile_pool(name="lnp", bufs=1) as lnp,
            ):
                ysq = lnp.tile([P, D_NODE, NT], BF16, name="ysq")
                nc.gpsimd.tensor_mul(out=ysq[:], in0=yt_sb[:], in1=yt_sb[:])
                w = D_NODE // 2
                cur = ysq
                while w >= 1:
                    nxt = lnp.tile([P, w, NT], BF16, name=f"vs{w}")
                    nc.vector.tensor_add(out=nxt[:], in0=cur[:, 0:w, :],
                                         in1=cur[:, w:2 * w, :])
                    cur = nxt
                    w //= 2
                # var = sumsq/64 - mu^2
                ss = lnp.tile([P, NT], F32, name="ss")
                nc.vector.tensor_scalar_mul(
                    ss[:], cur[:].rearrange("p o t -> p (o t)"), 1.0 / D_NODE)
                musq = lnp.tile([P, NT], F32, name="musq")
                nc.vector.tensor_mul(out=musq[:], in0=mu_sb[:], in1=mu_sb[:])
                nc.vector.tensor_sub(out=var_sb[:], in0=ss[:], in1=musq[:])
                sd = lnp.tile([P, NT], F32, name="sd")
                nc.scalar.activation(out=sd[:], in_=var_sb[:],
                                     func=mybir.ActivationFunctionType.Sqrt,
                                     bias=eps_sb[:])
                rstd = lnp.tile([P, NT], BF16, name="rstd")
                nc.vector.reciprocal(out=rstd[:], in_=sd[:])
                musd = lnp.tile([P, NT], BF16, name="musd")
                nc.vector.tensor_mul(out=musd[:], in0=mu_sb[:], in1=rstd[:])
                n1 = lnp.tile([P, D_NODE, NT], BF16, name="n1")
                rb = bass.AP(tensor=rstd[:].tensor, offset=rstd[:].offset,
                             ap=[rstd[:].ap[0], [0, D_NODE], [1, NT]])
                nc.vector.tensor_mul(out=n1[:], in0=yt_sb[:], in1=rb)
                mb = bass.AP(tensor=musd[:].tensor, offset=musd[:].offset,
                             ap=[musd[:].ap[0], [0, D_NODE], [1, NT]])
                yout = lnp.tile([P, D_NODE, NT], BF16, name="yout")
                nc.vector.tensor_sub(out=yout[:], in0=n1[:], in1=mb)
                if not ln_trivial:
                    gam = lnp.tile([P, D_NODE], F32, name="gam")
                    nc.sync.dma_start(
                        out=gam[:],
                        in_=bass.AP(tensor=gb[:].tensor, offset=0,
                                    ap=[[0, P], [1, D_NODE]]))
                    bet = lnp.tile([P, D_NODE], F32, name="bet")
                    nc.sync.dma_start(
                        out=bet[:],
                        in_=bass.AP(tensor=gb[:].tensor, offset=D_NODE,
                                    ap=[[0, P], [1, D_NODE]]))
                    gbc = bass.AP(tensor=gam[:].tensor, offset=gam[:].offset,
                                  ap=[gam[:].ap[0], [1, D_NODE], [0, NT]])
                    bbc = bass.AP(tensor=bet[:].tensor, offset=bet[:].offset,
                                  ap=[bet[:].ap[0], [1, D_NODE], [0, NT]])
                    nc.vector.tensor_mul(out=yout[:], in0=yout[:], in1=gbc)
                    nc.vector.tensor_add(out=yout[:], in0=yout[:], in1=bbc)
                nc.sync.dma_start(
                    out=y[:], in_=yout[:].rearrange("p f t -> p (f t)"))

    nc.compile()
    return nc


# ------------------------------------------------------------------ driver --
def kernel(**inputs) -> np.ndarray:
    per_core, node_lists, meta = _host_prep(**inputs)
    nc = _build_kernel(meta)
    res = run_bass_kernel_spmd(nc, per_core, core_ids=list(range(NCORES)))
    y_full = np.zeros((N, D_NODE), dtype=np.float32)
    for c in range(NCORES):
        yc = res.results[c]["y"].astype(np.float32)
        yc = yc.reshape(P, D_NODE, NT).transpose(2, 0, 1).reshape(NPC, D_NODE)
        nl = node_lists[c]
        real = nl >= 0
        y_full[nl[real]] = yc[real]
    return y_full
